# revision 1
# baseline (speedup 1.0000x reference)
"""BGAT attention kernel for Trainium2 (8 NeuronCores, batch-parallel).

Strategy (per core = one batch element):
  score[u,a,k] = (1/8) * sum_d av[k,d] * lrelu(S), S = (U+A+E)[u,a,(k,d)]
  Using lrelu(x) = 0.6x + 0.4|x|:
    score = T1 + sum_pos |S''| - sum_neg |S''|
  where S'' has per-column weights folded with 0.4/8*|av_d| (columns permuted
  so each head's positive-av columns sit in one padded uniform block, negative
  in another), and T1 = linear term via folded projection columns (exact).
  E-term weights ride a K=65 augmented matmul (ones row x U[u] row) so the
  per-user broadcast add is free; the A-term rides an identity matmul into the
  same PSUM accumulation.
  softmax needs no max-subtraction (scores are tiny by construction).
  Message sums commute with the edge projection:
    sum_a alpha*E = (sum_a alpha*edge) @ We   (and same over u)
  so phase 3 is small matmuls over natural-layout edge tiles.
"""

import math
from contextlib import ExitStack

import numpy as np

# ---- problem sizes (hardcoded from spec) ----
B = 8
FULL_CFG = dict(NU=256, NA=256, ED=64, UD=128, AD=128, H=8, HD=64)
SLOPE = 0.2


def make_cfg(NU, NA, ED, UD, AD, H, HD, av, UC=None):
    """Host-side layout metadata derived from av sign pattern."""
    cfg = dict(NU=NU, NA=NA, ED=ED, UD=UD, AD=AD, H=H, HD=HD)
    cfg["HH"] = H * HD
    scale = 1.0 / math.sqrt(HD)
    av = np.asarray(av, np.float32)
    pos_idx = [np.nonzero(av[k] >= 0)[0] for k in range(H)]
    neg_idx = [np.nonzero(av[k] < 0)[0] for k in range(H)]
    P_ = max(len(ix) for ix in pos_idx)
    N_ = max(len(ix) for ix in neg_idx)
    cfg["P_"], cfg["N_"] = P_, N_
    cfg["EXTC"] = H * P_ + H * N_ + H
    cfg["pos_idx"], cfg["neg_idx"] = pos_idx, neg_idx
    cfg["scale"] = scale
    cfg["NAH"] = (NA + 127) // 128  # number of 128-wide antenna chunks
    cfg["ACH"] = min(128, NA)
    cfg["UC"] = min(128, NU) if UC is None else UC
    cfg["NUC"] = NU // cfg["UC"]  # number of user chunks
    assert NU % 8 == 0
    cfg["NG"] = NU // 8  # softmax groups of 8 users
    return cfg


def prep_weights(Wu, Wa, We, av, Wres, cfg):
    """Build folded/permuted weight blocks. Returns dict of np arrays."""
    H, HD, ED, UD, AD = cfg["H"], cfg["HD"], cfg["ED"], cfg["UD"], cfg["AD"]
    P_, N_, EXTC, HH = cfg["P_"], cfg["N_"], cfg["EXTC"], cfg["HH"]
    scale = cfg["scale"]
    Wu, Wa, We = (np.asarray(x, np.float32) for x in (Wu, Wa, We))
    av = np.asarray(av, np.float32)
    Wres = np.asarray(Wres, np.float32)

    wu_big = np.zeros((UD, EXTC + HH), np.float32)
    wa_big = np.zeros((AD, EXTC + HH), np.float32)
    we_big = np.zeros((ED, EXTC + HH), np.float32)
    for k in range(H):
        for i, d in enumerate(cfg["pos_idx"][k]):
            c = 0.4 * scale * abs(av[k, d])
            col = k * P_ + i
            wu_big[:, col] = Wu[k][:, d] * c
            wa_big[:, col] = Wa[k][:, d] * c
            we_big[:, col] = We[k][:, d] * c
        for i, d in enumerate(cfg["neg_idx"][k]):
            c = 0.4 * scale * abs(av[k, d])
            col = H * P_ + k * N_ + i
            wu_big[:, col] = Wu[k][:, d] * c
            wa_big[:, col] = Wa[k][:, d] * c
            we_big[:, col] = We[k][:, d] * c
        # T1 (linear) columns: W @ (0.6*scale*av_k)
        t1w = 0.6 * scale * av[k]
        col = H * P_ + H * N_ + k
        wu_big[:, col] = Wu[k] @ t1w
        wa_big[:, col] = Wa[k] @ t1w
        we_big[:, col] = We[k] @ t1w
        # raw blocks for message matmuls
        wu_big[:, EXTC + k * HD : EXTC + (k + 1) * HD] = Wu[k]
        wa_big[:, EXTC + k * HD : EXTC + (k + 1) * HD] = Wa[k]
        we_big[:, EXTC + k * HD : EXTC + (k + 1) * HD] = We[k]

    ident = np.eye(128, dtype=np.float32)
    return dict(wu_big=wu_big, wa_big=wa_big, we_big=we_big, wres=Wres,
                ident=ident)


def build_bgat(ctx: ExitStack, tc, outs, ins, cfg):
    """Emit the Tile program. outs/ins: dicts name->AP."""
    import concourse.bass as bass
    import concourse.mybir as mybir

    nc = tc.nc
    f32 = mybir.dt.float32
    AX = mybir.AxisListType.X
    ADD = mybir.AluOpType.add
    EXPF = mybir.ActivationFunctionType.Exp

    NU, NA, ED, UD, AD = cfg["NU"], cfg["NA"], cfg["ED"], cfg["UD"], cfg["AD"]
    H, HD, HH = cfg["H"], cfg["HD"], cfg["HH"]
    P_, N_, EXTC = cfg["P_"], cfg["N_"], cfg["EXTC"]
    NAH, ACH, UC, NUC, NG = cfg["NAH"], cfg["ACH"], cfg["UC"], cfg["NUC"], cfg["NG"]
    HIDDEN = HH
    POSW, NEGW = H * P_, H * N_

    edge = ins["edge"]      # [NU*NA, ED]
    user = ins["user"]      # [NU, UD]
    ant = ins["ant"]        # [NA, AD]
    wu_big_d = ins["wu_big"]
    wa_big_d = ins["wa_big"]
    we_big_d = ins["we_big"]
    wres_d = ins["wres"]
    ident_d = ins["ident"]
    user_out = outs["user_out"]  # [NU, HIDDEN]
    ant_out = outs["ant_out"]    # [NA, HIDDEN]

    # x-major chunked view of edge: chunk c has 128 consecutive (u,a) rows
    CH = ACH  # rows per chunk (128 at full size)
    n_chunks_per_u = NAH
    edge_x = edge.rearrange("(c p) e -> c p e", p=CH)
    # u-major view for ant-side: partition = user
    edge_u = edge.rearrange("(j p a) e -> j p (a e)", p=UC, a=NA)

    consts = ctx.enter_context(tc.tile_pool(name="consts", bufs=1))

    # ---------- persistent SBUF tensors ----------
    ident_sb = consts.tile([128, 128], f32)
    nc.sync.dma_start(ident_sb[:], ident_d[:, :])
    wu_big_sb = consts.tile([UD, EXTC + HH], f32)
    nc.sync.dma_start(wu_big_sb[:], wu_big_d[:, :])
    wa_big_sb = consts.tile([AD, EXTC + HH], f32)
    nc.sync.dma_start(wa_big_sb[:], wa_big_d[:, :])
    we_big_sb = consts.tile([ED, EXTC + HH], f32)
    nc.sync.dma_start(we_big_sb[:], we_big_d[:, :])
    wres_sb = consts.tile([UD, HIDDEN], f32)
    nc.sync.dma_start(wres_sb[:], wres_d[:, :])

    ones_col = consts.tile([128, 1], f32)
    nc.gpsimd.memset(ones_col[:], 1.0)
    ones_row = consts.tile([1, 128], f32)
    nc.gpsimd.memset(ones_row[:], 1.0)

    U_big = consts.tile([UC, NUC, EXTC + HH], f32)
    A_big = consts.tile([ACH, NAH, EXTC + HH], f32)
    userT = consts.tile([UD, NU], f32)
    antT = consts.tile([AD, NA], f32)
    # alpha layouts: v3 = antenna-major, head-outer; v2 = user-major
    alpha_v3 = consts.tile([ACH, NAH, H, NU], f32)
    alpha_v2 = consts.tile([UC, NUC, H, NA], f32)
    ew_all = consts.tile([ED, NU, H], f32)
    ewa_all = consts.tile([ED, NA, H], f32)

    # combo rhs tiles (rows 0..ED-1 = we_big ext cols, row ED = per-user U row)
    combo0 = consts.tile([ED + 1, EXTC], f32)
    combo1 = consts.tile([ED + 1, EXTC], f32)
    combos = [combo0, combo1]
    for cb in combos:
        nc.gpsimd.dma_start(cb[0:ED, :], we_big_d[:, 0:EXTC])

    # ---------- precompute: transposes and U/A projections ----------
    with tc.tile_pool(name="pre_sb", bufs=2) as pre_sb, \
         tc.tile_pool(name="pre_ps", bufs=2, space="PSUM") as pre_ps:
        # user/ant feature tiles and transposes
        for (feat, T_sb, n, fd) in ((user, userT, NU, UD), (ant, antT, NA, AD)):
            fv = feat.rearrange("(j p) f -> j p f", p=min(128, n))
            for j in range(fv.shape[0]):
                p = fv.shape[1]
                ft = pre_sb.tile([p, fd], f32, tag="ft")
                nc.sync.dma_start(ft[:], fv[j])
                pt = pre_ps.tile([fd, p], f32, tag="pt")
                nc.tensor.transpose(pt[:], ft[:], ident_sb[0:p, 0:p])
                nc.scalar.copy(T_sb[:, j * p : j * p + p], pt[:])
        # U_big / A_big
        for (T_sb, big, nchunk, pc, fd) in (
            (userT, U_big, NUC, UC, UD),
            (antT, A_big, NAH, ACH, AD),
        ):
            w_sb = wu_big_sb if big is U_big else wa_big_sb
            for j in range(nchunk):
                for c0 in range(0, EXTC + HH, 512):
                    c1 = min(c0 + 512, EXTC + HH)
                    ps = pre_ps.tile([pc, 512], f32, tag="proj")
                    nc.tensor.matmul(ps[:, 0 : c1 - c0],
                                     T_sb[:, j * pc : j * pc + pc],
                                     w_sb[:, c0:c1], start=True, stop=True)
                    nc.scalar.copy(big[:, j, c0:c1], ps[:, 0 : c1 - c0])

    # ---------- pass 1: scores + softmax + user-side weighted edge sums ----
    # psum_misc bank layout (per group of 8 users):
    T1_OFF = 0                      # [128, NAH*8*H]
    SUM_OFF = T1_OFF + NAH * 8 * H  # [1, 8*H]
    RB_OFF = SUM_OFF + 8 * H        # [128, 8*H]
    EW_OFF = RB_OFF + 8 * H         # [ED, 8*H]
    assert EW_OFF + 8 * H <= 512

    with tc.tile_pool(name="edge_pool", bufs=6 * NAH + 2 * 8 * NAH) as edge_pool, \
         tc.tile_pool(name="p1_sb", bufs=3) as p1_sb, \
         tc.tile_pool(name="p1_stage", bufs=2) as p1_stage, \
         tc.tile_pool(name="ps_pos", bufs=2, space="PSUM") as ps_pos_pool, \
         tc.tile_pool(name="ps_neg", bufs=2, space="PSUM") as ps_neg_pool, \
         tc.tile_pool(name="ps_tp", bufs=2, space="PSUM") as ps_tp_pool, \
         tc.tile_pool(name="ps_misc", bufs=2, space="PSUM") as ps_misc_pool:

        # chunk list per group, in emission order, processed in sub-batches
        # of 4 (4 transposes share one psum bank + one batched copy)
        assert (8 * NAH) % 4 == 0
        for g in range(NG):
            misc = ps_misc_pool.tile([128, 512], f32, tag="misc")
            stage_P = p1_stage.tile([ACH, NAH * 8, H], f32, tag="sP")
            stage_N = p1_stage.tile([ACH, NAH * 8, H], f32, tag="sN")
            edge_tiles = {}
            chunks = [(ui, h) for ui in range(8) for h in range(NAH)]
            for u4 in range(0, len(chunks), 4):
                batch = chunks[u4 : u4 + 4]
                tp = ps_tp_pool.tile([ED, 512], f32, tag="tp")
                edT4 = p1_sb.tile([ED + 1, 512], f32, tag="edT4")
                nc.vector.memset(edT4[ED : ED + 1, :], 1.0)
                for q, (ui, h) in enumerate(batch):
                    u = g * 8 + ui
                    c = u * n_chunks_per_u + h
                    et = edge_pool.tile([CH, ED], f32, tag="edge")
                    nc.sync.dma_start(et[:], edge_x[c])
                    edge_tiles[(ui, h)] = et
                    nc.tensor.transpose(tp[:, q * 128 : q * 128 + CH], et[:],
                                        ident_sb[0:CH, 0:CH])
                for q, (ui, h) in enumerate(batch):
                    if h == 0:
                        u = g * 8 + ui
                        cb = combos[u % 2]
                        # per-user U row into combo row ED
                        nc.gpsimd.dma_start(
                            cb[ED : ED + 1, :],
                            U_big[u % UC : u % UC + 1, u // UC, 0:EXTC])
                nc.scalar.copy(edT4[0:ED, :], tp[:, :])
                for q, (ui, h) in enumerate(batch):
                    u = g * 8 + ui
                    cb = combos[u % 2]
                    sidx = h * 8 + ui
                    t1s = misc[0:CH, T1_OFF + sidx * H : T1_OFF + (sidx + 1) * H]
                    lhs = edT4[0 : ED + 1, q * 128 : q * 128 + CH]
                    ps_pos = ps_pos_pool.tile([CH, 512], f32, tag="pos")
                    ps_neg = ps_neg_pool.tile([CH, 512], f32, tag="neg")
                    # E+U into psum (K=ED+1 augmented), then A via identity mm
                    nc.tensor.matmul(ps_pos[:, 0:POSW], lhs, cb[:, 0:POSW],
                                     start=True, stop=False)
                    nc.tensor.matmul(ps_neg[:, 0:NEGW], lhs,
                                     cb[:, POSW : POSW + NEGW],
                                     start=True, stop=False)
                    nc.tensor.matmul(t1s, lhs, cb[:, POSW + NEGW : EXTC],
                                     start=True, stop=False)
                    nc.tensor.matmul(ps_pos[:, 0:POSW], ident_sb[0:ACH, 0:ACH],
                                     A_big[:, h, 0:POSW], start=False, stop=True)
                    nc.tensor.matmul(ps_neg[:, 0:NEGW], ident_sb[0:ACH, 0:ACH],
                                     A_big[:, h, POSW : POSW + NEGW],
                                     start=False, stop=True)
                    nc.tensor.matmul(t1s, ident_sb[0:ACH, 0:ACH],
                                     A_big[:, h, POSW + NEGW : EXTC],
                                     start=False, stop=True)
                    # |.| reduces
                    nc.vector.tensor_reduce(
                        stage_P[:, sidx, :],
                        ps_pos[:, 0:POSW].rearrange("p (k d) -> p k d", d=P_),
                        axis=AX, op=ADD, apply_absolute_value=True)
                    nc.vector.tensor_reduce(
                        stage_N[:, sidx, :],
                        ps_neg[:, 0:NEGW].rearrange("p (k d) -> p k d", d=N_),
                        axis=AX, op=ADD, apply_absolute_value=True)

            # ---- group softmax ----
            # score_g memory order (h, u, k); exp_g memory order (h, k, u)
            gsz = NAH * 8 * H
            score_g = p1_sb.tile([ACH, gsz], f32, tag="score", bufs=4)
            nc.vector.tensor_sub(score_g[:],
                                 stage_P[:].rearrange("p a b -> p (a b)"),
                                 stage_N[:].rearrange("p a b -> p (a b)"))
            nc.vector.tensor_add(score_g[:], score_g[:],
                                 misc[0:ACH, T1_OFF : T1_OFF + gsz])
            exp_g = p1_sb.tile([ACH, gsz], f32, tag="expg", bufs=6)
            nc.scalar.activation(
                exp_g[:].rearrange("p (a c b) -> p a b c", a=NAH, c=H),
                score_g[:].rearrange("p (a b c) -> p a b c", a=NAH, b=8),
                EXPF)
            for h in range(NAH):
                nc.tensor.matmul(
                    misc[0:1, SUM_OFF : SUM_OFF + 8 * H], ones_col[0:ACH, :],
                    exp_g[:, h * 8 * H : (h + 1) * 8 * H],
                    start=(h == 0), stop=(h == NAH - 1))
            rec = p1_sb.tile([1, 8 * H], f32, tag="rec", bufs=4)
            nc.vector.reciprocal(rec[:], misc[0:1, SUM_OFF : SUM_OFF + 8 * H])
            nc.tensor.matmul(misc[0:128, RB_OFF : RB_OFF + 8 * H],
                             ones_row[:, 0:128], rec[:], start=True, stop=True)
            # alpha (normalized), kept in flat group tile + scattered to v3
            for h in range(NAH):
                sl = exp_g[:, h * 8 * H : (h + 1) * 8 * H]
                nc.vector.tensor_mul(sl, sl,
                                     misc[0:ACH, RB_OFF : RB_OFF + 8 * H])
                nc.vector.tensor_copy(
                    alpha_v3[:, h, :, g * 8 : g * 8 + 8],
                    sl.rearrange("p (k u) -> p k u", k=H))
            # ---- user-side weighted edge sums ----
            for ui in range(8):
                u = g * 8 + ui
                for h in range(NAH):
                    al_u = exp_g[:, h * 8 * H : (h + 1) * 8 * H].rearrange(
                        "p (k u) -> p k u", k=H)[:, :, ui]
                    nc.tensor.matmul(
                        misc[0:ED, EW_OFF + ui * H : EW_OFF + (ui + 1) * H],
                        edge_tiles[(ui, h)][:], al_u,
                        start=(h == 0), stop=(h == NAH - 1))
            nc.vector.tensor_copy(
                ew_all[:, g * 8 : g * 8 + 8, :].rearrange("p a b -> p (a b)"),
                misc[0:ED, EW_OFF : EW_OFF + 8 * H])

    # ---------- pass 3: ant-side sums and outputs ----------
    with tc.tile_pool(name="p3_sb", bufs=3) as p3_sb, \
         tc.tile_pool(name="p3_ps", bufs=2, space="PSUM") as p3_ps, \
         tc.tile_pool(name="po_ps", bufs=2, space="PSUM") as po_ps:
        # alpha_v2 (user-major) via direct [128,128] transposes of alpha_v3
        for j in range(NUC):
            for k in range(H):
                for h in range(NAH):
                    pt2 = p3_ps.tile([UC, 512], f32, tag="pt2")
                    nc.tensor.transpose(
                        pt2[:, 0:ACH],
                        alpha_v3[:, h, k, j * UC : (j + 1) * UC],
                        ident_sb[0:ACH, 0:ACH])
                    nc.scalar.copy(
                        alpha_v2[:, j, k, h * ACH : (h + 1) * ACH],
                        pt2[0:UC, 0:ACH])
        # ant-side weighted edge sums (contract over users); edge streamed
        # u-major in 8-antenna slabs
        edge_u4 = edge.rearrange("(j p a) e -> j p a e", p=UC, a=NA)
        for ag in range(NA // 8):
            ev = p3_sb.tile([UC, NUC, 8, ED], f32, tag="ev")
            for j in range(NUC):
                for ap2 in range(0, 8, 2):
                    nc.sync.dma_start(
                        ev[:, j, ap2 : ap2 + 2, :],
                        edge_u4[j, :, ag * 8 + ap2 : ag * 8 + ap2 + 2, :])
            pe = p3_ps.tile([ED, 512], f32, tag="pewa")
            for ai in range(8):
                a = ag * 8 + ai
                for j in range(NUC):
                    nc.tensor.matmul(
                        pe[:, ai * H : (ai + 1) * H],
                        ev[:, j, ai, :], alpha_v2[:, j, :, a],
                        start=(j == 0), stop=(j == NUC - 1))
            nc.vector.tensor_copy(
                ewa_all[:, ag * 8 : ag * 8 + 8, :].rearrange("p a b -> p (a b)"),
                pe[:, 0 : 8 * H])
        # user_out = concat_k(alpha@A_k + ew@We_k) + user@Wres
        uo_v = user_out.rearrange("(j p) d -> j p d", p=UC)
        for j in range(NUC):
            po = po_ps.tile([UC, HIDDEN], f32, tag="puo")
            for k in range(H):
                nc.tensor.matmul(po[:, k * HD : (k + 1) * HD],
                                 userT[:, j * UC : j * UC + UC],
                                 wres_sb[:, k * HD : (k + 1) * HD],
                                 start=True, stop=False)
                for h in range(NAH):
                    nc.tensor.matmul(
                        po[:, k * HD : (k + 1) * HD],
                        alpha_v3[:, h, k, j * UC : j * UC + UC],
                        A_big[:, h, EXTC + k * HD : EXTC + (k + 1) * HD],
                        start=False, stop=False)
                nc.tensor.matmul(
                    po[:, k * HD : (k + 1) * HD],
                    ew_all[:, j * UC : j * UC + UC, k],
                    we_big_sb[:, EXTC + k * HD : EXTC + (k + 1) * HD],
                    start=False, stop=True)
            ob = p3_sb.tile([UC, HIDDEN], f32, tag="ob")
            nc.scalar.copy(ob[:], po[:])
            nc.sync.dma_start(uo_v[j], ob[:])
        # ant_out = concat_k(alpha^T@U_k + ewa@We_k)
        ao_v = ant_out.rearrange("(i p) d -> i p d", p=ACH)
        for i in range(NA // ACH):
            po = po_ps.tile([ACH, HIDDEN], f32, tag="pao")
            for k in range(H):
                for j in range(NUC):
                    nc.tensor.matmul(
                        po[:, k * HD : (k + 1) * HD],
                        alpha_v2[:, j, k, i * ACH : (i + 1) * ACH],
                        U_big[:, j, EXTC + k * HD : EXTC + (k + 1) * HD],
                        start=(j == 0), stop=False)
                nc.tensor.matmul(
                    po[:, k * HD : (k + 1) * HD],
                    ewa_all[:, i * ACH : (i + 1) * ACH, k],
                    we_big_sb[:, EXTC + k * HD : EXTC + (k + 1) * HD],
                    start=False, stop=True)
            ob = p3_sb.tile([ACH, HIDDEN], f32, tag="ob2")
            nc.scalar.copy(ob[:], po[:])
            nc.sync.dma_start(ao_v[i], ob[:])


# ---------------------------------------------------------------------------
_CACHE = {}


def _get_nc(cfg):
    key = "nc"
    if key in _CACHE:
        return _CACHE[key]
    import concourse.bacc as bacc
    import concourse.mybir as mybir
    import concourse.tile as tile

    f32 = mybir.dt.float32
    nc = bacc.Bacc("TRN2", target_bir_lowering=False, debug=False)
    NU, NA, ED, UD, AD = cfg["NU"], cfg["NA"], cfg["ED"], cfg["UD"], cfg["AD"]
    EXTC, HH = cfg["EXTC"], cfg["HH"]
    ins = {
        "edge": nc.dram_tensor("edge", [NU * NA, ED], f32, kind="ExternalInput").ap(),
        "user": nc.dram_tensor("user", [NU, UD], f32, kind="ExternalInput").ap(),
        "ant": nc.dram_tensor("ant", [NA, AD], f32, kind="ExternalInput").ap(),
        "wu_big": nc.dram_tensor("wu_big", [UD, EXTC + HH], f32, kind="ExternalInput").ap(),
        "wa_big": nc.dram_tensor("wa_big", [AD, EXTC + HH], f32, kind="ExternalInput").ap(),
        "we_big": nc.dram_tensor("we_big", [ED, EXTC + HH], f32, kind="ExternalInput").ap(),
        "wres": nc.dram_tensor("wres", [UD, HH], f32, kind="ExternalInput").ap(),
        "ident": nc.dram_tensor("ident", [128, 128], f32, kind="ExternalInput").ap(),
    }
    outs = {
        "user_out": nc.dram_tensor("user_out", [NU, HH], f32, kind="ExternalOutput").ap(),
        "ant_out": nc.dram_tensor("ant_out", [NA, HH], f32, kind="ExternalOutput").ap(),
    }
    with tile.TileContext(nc) as tc:
        with ExitStack() as ctx:
            build_bgat(ctx, tc, outs, ins, cfg)
    nc.finalize()
    _CACHE[key] = nc
    return nc


_LAST_RES = {}


def kernel(user_feats, ant_feats, edge_feats, Wu, Wa, We, av, Wres,
           _trace=False):
    from concourse.bass_utils import run_bass_kernel_spmd

    user_feats = np.asarray(user_feats, np.float32)
    ant_feats = np.asarray(ant_feats, np.float32)
    edge_feats = np.asarray(edge_feats, np.float32)
    cfg = make_cfg(**FULL_CFG, av=av)
    wd = prep_weights(Wu, Wa, We, av, Wres, cfg)
    nc = _get_nc(cfg)
    NU, NA, ED = cfg["NU"], cfg["NA"], cfg["ED"]
    in_maps = []
    for b in range(B):
        in_maps.append({
            "edge": np.ascontiguousarray(edge_feats[b].reshape(NU * NA, ED)),
            "user": np.ascontiguousarray(user_feats[b]),
            "ant": np.ascontiguousarray(ant_feats[b]),
            "wu_big": wd["wu_big"], "wa_big": wd["wa_big"],
            "we_big": wd["we_big"], "wres": wd["wres"], "ident": wd["ident"],
        })
    res = run_bass_kernel_spmd(nc, in_maps, core_ids=list(range(B)),
                               trace=_trace)
    _LAST_RES["res"] = res
    user_out = np.stack([res.results[b]["user_out"] for b in range(B)])
    ant_out = np.stack([res.results[b]["ant_out"] for b in range(B)])
    return (user_out, ant_out)



# revision 3
# speedup vs baseline: 1.2021x; 1.2021x over previous
"""BGAT attention kernel v2 for Trainium2 (8 NeuronCores, batch-parallel).

Changes vs v1: all matmul operands bf16 (4x PE throughput per the cost
model's 4-cycles-per-row fp32 penalty), edge shipped/loaded as bf16
(half wire + HBM bytes), batched edge DMAs, single fused abs-reduce per
tile with uniform head padding D, ACT-engine combo-row assembly (no
gpsimd SBUF-SBUF DMAs), fattened output matmuls.

Math (per core = one batch element b):
  score[u,a,k] = scale * av_k . lrelu(U_k[u] + A_k[a] + E_k[u,a])
  with lrelu(x) = 0.6x + 0.4|x|:
    score = T1 + sum_pos |S''| - sum_neg |S''|
  S''[., c] columns hold (U+A+E) projected through per-column weights
  folded with 0.4*scale*|av_d|; T1 is the exact linear term.
  E-term rides a K=65 matmul (transposed edge tile + ones row x U-row);
  A-term rides an identity matmul into the same PSUM accumulation.
  alpha = softmax(score) over antennas (no max-subtraction needed:
  scores are tiny by construction).
  Messages contract edge with alpha BEFORE projecting:
    sum_a alpha*E = (sum_a alpha*edge) @ We  (same over u).
"""

import math
from contextlib import ExitStack

import numpy as np
import ml_dtypes

BF16 = ml_dtypes.bfloat16

# ---- problem sizes (hardcoded from spec) ----
B = 8
FULL_CFG = dict(NU=256, NA=256, ED=64, UD=128, AD=128, H=8, HD=64)
SLOPE = 0.2


def make_cfg(NU, NA, ED, UD, AD, H, HD, av):
    cfg = dict(NU=NU, NA=NA, ED=ED, UD=UD, AD=AD, H=H, HD=HD)
    cfg["HH"] = H * HD
    scale = 1.0 / math.sqrt(HD)
    av = np.asarray(av, np.float32)
    pos_idx = [np.nonzero(av[k] >= 0)[0] for k in range(H)]
    neg_idx = [np.nonzero(av[k] < 0)[0] for k in range(H)]
    D = max(max(len(ix) for ix in pos_idx), max(len(ix) for ix in neg_idx))
    cfg["D"] = D
    cfg["C0"] = 2 * H * D          # score cols: pos block | neg block
    cfg["CT"] = cfg["C0"] + H      # + T1 cols
    cfg["pos_idx"], cfg["neg_idx"] = pos_idx, neg_idx
    cfg["scale"] = scale
    cfg["NAH"] = (NA + 127) // 128
    cfg["ACH"] = min(128, NA)
    cfg["UC"] = min(128, NU)
    cfg["NUC"] = NU // cfg["UC"]
    assert NU % 8 == 0
    cfg["NG"] = NU // 8
    return cfg


def prep_weights(Wu, Wa, We, av, Wres, cfg):
    """Folded/permuted weight blocks in bf16. Column layout (CT+HH wide):
    [k*D+i : pos] [H*D + k*D+i : neg] [2*H*D + k : T1] [CT + k*HD+d : raw]"""
    H, HD, ED, UD, AD = cfg["H"], cfg["HD"], cfg["ED"], cfg["UD"], cfg["AD"]
    D, C0, CT, HH = cfg["D"], cfg["C0"], cfg["CT"], cfg["HH"]
    scale = cfg["scale"]
    Wu, Wa, We = (np.asarray(x, np.float32) for x in (Wu, Wa, We))
    av = np.asarray(av, np.float32)
    Wres = np.asarray(Wres, np.float32)

    wu_big = np.zeros((UD, CT + HH), np.float32)
    wa_big = np.zeros((AD, CT + HH), np.float32)
    we_big = np.zeros((ED, CT + HH), np.float32)
    for k in range(H):
        for i, d in enumerate(cfg["pos_idx"][k]):
            c = 0.4 * scale * abs(av[k, d])
            col = k * D + i
            wu_big[:, col] = Wu[k][:, d] * c
            wa_big[:, col] = Wa[k][:, d] * c
            we_big[:, col] = We[k][:, d] * c
        for i, d in enumerate(cfg["neg_idx"][k]):
            c = 0.4 * scale * abs(av[k, d])
            col = H * D + k * D + i
            wu_big[:, col] = Wu[k][:, d] * c
            wa_big[:, col] = Wa[k][:, d] * c
            we_big[:, col] = We[k][:, d] * c
        t1w = 0.6 * scale * av[k]
        col = C0 + k
        wu_big[:, col] = Wu[k] @ t1w
        wa_big[:, col] = Wa[k] @ t1w
        we_big[:, col] = We[k] @ t1w
        wu_big[:, CT + k * HD : CT + (k + 1) * HD] = Wu[k]
        wa_big[:, CT + k * HD : CT + (k + 1) * HD] = Wa[k]
        we_big[:, CT + k * HD : CT + (k + 1) * HD] = We[k]

    ident = np.eye(128, dtype=np.float32)
    return dict(wu_big=wu_big.astype(BF16), wa_big=wa_big.astype(BF16),
                we_big=we_big.astype(BF16), wres=Wres.astype(BF16),
                ident=ident.astype(BF16))


def build_bgat(ctx: ExitStack, tc, outs, ins, cfg):
    import concourse.bass as bass
    import concourse.mybir as mybir

    nc = tc.nc
    f32 = mybir.dt.float32
    bf16 = mybir.dt.bfloat16
    AX = mybir.AxisListType.X
    ADD = mybir.AluOpType.add
    EXPF = mybir.ActivationFunctionType.Exp

    NU, NA, ED, UD, AD = cfg["NU"], cfg["NA"], cfg["ED"], cfg["UD"], cfg["AD"]
    H, HD, HH = cfg["H"], cfg["HD"], cfg["HH"]
    D, C0, CT = cfg["D"], cfg["C0"], cfg["CT"]
    NAH, ACH, UC, NUC, NG = cfg["NAH"], cfg["ACH"], cfg["UC"], cfg["NUC"], cfg["NG"]
    HIDDEN = HH

    edge = ins["edge"]      # [NU*NA, ED] bf16
    user = ins["user"]      # [NU, UD] bf16
    ant = ins["ant"]        # [NA, AD] bf16
    wu_big_d = ins["wu_big"]
    wa_big_d = ins["wa_big"]
    we_big_d = ins["we_big"]
    wres_d = ins["wres"]
    ident_d = ins["ident"]
    user_out = outs["user_out"]  # [NU, HIDDEN] f32
    ant_out = outs["ant_out"]    # [NA, HIDDEN] f32

    # edge views
    # pass-1: per user u, antenna-major tiles [128a, 64e]
    edge_p1 = edge.rearrange("(u h p) e -> p u h e", h=NAH, p=ACH)
    # pass-3: user-major slabs [128u, 8a, 64e]
    edge_p3 = edge.rearrange("(j p a) e -> j p a e", p=UC, a=NA)

    consts = ctx.enter_context(tc.tile_pool(name="consts", bufs=1))

    # ---------- persistent SBUF tensors ----------
    ident_sb = consts.tile([128, 128], bf16)
    nc.sync.dma_start(ident_sb[:], ident_d[:, :])
    wu_big_sb = consts.tile([UD, CT + HH], bf16)
    nc.sync.dma_start(wu_big_sb[:], wu_big_d[:, :])
    wa_big_sb = consts.tile([AD, CT + HH], bf16)
    nc.sync.dma_start(wa_big_sb[:], wa_big_d[:, :])
    we_big_sb = consts.tile([ED, CT + HH], bf16)
    nc.sync.dma_start(we_big_sb[:], we_big_d[:, :])
    wres_sb = consts.tile([UD, HIDDEN], bf16)
    nc.sync.dma_start(wres_sb[:], wres_d[:, :])

    ones_col = consts.tile([128, 1], f32)
    nc.gpsimd.memset(ones_col[:], 1.0)
    ones_row = consts.tile([1, 128], f32)
    nc.gpsimd.memset(ones_row[:], 1.0)

    U_big = consts.tile([UC, NUC, CT + HH], bf16)
    A_big = consts.tile([ACH, NAH, CT + HH], bf16)
    userT = consts.tile([UD, NU], bf16)
    antT = consts.tile([AD, NA], bf16)
    alpha_v3 = consts.tile([ACH, NAH, H, NU], bf16)   # [a, h, k, u]
    alpha_v2 = consts.tile([UC, NUC, H, NA], bf16)    # [u, j, k, a]
    ew_all = consts.tile([ED, NU, H], bf16)
    ewa_all = consts.tile([ED, NA, H], bf16)

    # edT ring: transposed edge tiles [65, 4*128] (4 chunks each), row 64=1
    N_EDT = 8
    edT_ring = []
    for i in range(N_EDT):
        edT_i = consts.tile([ED + 1, 512], bf16, tag=f"edT{i}", name=f"edT{i}")
        edT_ring.append(edT_i)
    for t in edT_ring:
        nc.vector.memset(t[ED : ED + 1, :], 1.0)
    # combo ring: [65, CT]; rows 0:64 = we_big[:, 0:CT] const, row 64 = U row
    N_CB = 4
    combos = []
    for i in range(N_CB):
        cb_i = consts.tile([ED + 1, CT], bf16, tag=f"cb{i}", name=f"cb{i}")
        combos.append(cb_i)
    for cb in combos:
        nc.sync.dma_start(cb[0:ED, :], we_big_d[:, 0:CT])

    # ---------- pass 0: transposes and U/A projections ----------
    with tc.tile_pool(name="pre_sb", bufs=2) as pre_sb, \
         tc.tile_pool(name="pre_ps", bufs=2, space="PSUM") as pre_ps:
        for (feat, T_sb, n, fd) in ((user, userT, NU, UD), (ant, antT, NA, AD)):
            fv = feat.rearrange("(j p) f -> j p f", p=min(128, n))
            for j in range(fv.shape[0]):
                p = fv.shape[1]
                ft = pre_sb.tile([p, fd], bf16, tag="ft")
                nc.sync.dma_start(ft[:], fv[j])
                pt = pre_ps.tile([fd, p], bf16, tag="pt")
                nc.tensor.transpose(pt[:], ft[:], ident_sb[0:p, 0:p])
                nc.scalar.copy(T_sb[:, j * p : j * p + p], pt[:])
        for (T_sb, big, nchunk, pc, fd) in (
            (userT, U_big, NUC, UC, UD),
            (antT, A_big, NAH, ACH, AD),
        ):
            w_sb = wu_big_sb if big is U_big else wa_big_sb
            for j in range(nchunk):
                for c0 in range(0, CT + HH, 512):
                    c1 = min(c0 + 512, CT + HH)
                    ps = pre_ps.tile([pc, 512], f32, tag="proj")
                    nc.tensor.matmul(ps[:, 0 : c1 - c0],
                                     T_sb[:, j * pc : j * pc + pc],
                                     w_sb[:, c0:c1], start=True, stop=True)
                    nc.scalar.copy(big[:, j, c0:c1], ps[:, 0 : c1 - c0])

    # ---------- pass 1: scores + softmax + user-side weighted edge sums ----
    # misc psum bank layout (per group of 8 users):
    T1_OFF = 0                       # [128, 16*8]
    SUM_OFF = T1_OFF + NAH * 8 * H   # [1, 64]
    RB_OFF = SUM_OFF + 8 * H         # [128, 64]
    EW_OFF = RB_OFF + 8 * H          # [64, 64]
    assert EW_OFF + 8 * H <= 512

    n_tp_batches = 0
    with tc.tile_pool(name="nat_pool", bufs=6) as nat_pool, \
         tc.tile_pool(name="p1_sb", bufs=3) as p1_sb, \
         tc.tile_pool(name="p1_stage", bufs=2) as p1_stage, \
         tc.tile_pool(name="ps_score", bufs=2, space="PSUM") as ps_score_pool, \
         tc.tile_pool(name="ps_tp", bufs=2, space="PSUM") as ps_tp_pool, \
         tc.tile_pool(name="ps_misc", bufs=2, space="PSUM") as ps_misc_pool:

        for g in range(NG):
            misc = ps_misc_pool.tile([128, 512], f32, tag="misc")
            stage = p1_stage.tile([ACH, NAH * 8, 2 * H], f32, tag="st")
            # natural-layout edge tiles for this group: 2 DMAs x 4 users
            nats = []
            for q in range(2):
                natt = nat_pool.tile([ACH, 4, NAH, ED], bf16, tag="nat")
                u0 = g * 8 + q * 4
                nc.sync.dma_start(natt[:], edge_p1[:, u0 : u0 + 4])
                nats.append(natt)

            chunks = [(ui, h) for ui in range(8) for h in range(NAH)]
            for u4 in range(0, len(chunks), 4):
                batch = chunks[u4 : u4 + 4]
                edT = edT_ring[n_tp_batches % N_EDT]
                n_tp_batches += 1
                tp = ps_tp_pool.tile([ED, 512], bf16, tag="tp")
                for q, (ui, h) in enumerate(batch):
                    nc.tensor.transpose(
                        tp[:, q * 128 : q * 128 + ACH],
                        nats[ui // 4][:, ui % 4, h, :],
                        ident_sb[0:ACH, 0:ACH])
                for q, (ui, h) in enumerate(batch):
                    if h == 0:
                        u = g * 8 + ui
                        cb = combos[u % N_CB]
                        nc.sync.dma_start(
                            cb[ED : ED + 1, :],
                            U_big[u % UC : u % UC + 1, u // UC, 0:CT])
                nc.scalar.copy(edT[0:ED, :], tp[:, :])
                for q, (ui, h) in enumerate(batch):
                    u = g * 8 + ui
                    cb = combos[u % N_CB]
                    sidx = h * 8 + ui
                    lhs = edT[0 : ED + 1, q * 128 : q * 128 + ACH]
                    sc = ps_score_pool.tile([ACH, C0], f32, tag="sc")
                    t1s = misc[0:ACH, T1_OFF + sidx * H : T1_OFF + (sidx + 1) * H]
                    # E+U into psum (K=65 augmented), then A via identity mm
                    nc.tensor.matmul(sc[:, 0:512], lhs, cb[:, 0:512],
                                     start=True, stop=False)
                    if C0 > 512:
                        nc.tensor.matmul(sc[:, 512:C0], lhs, cb[:, 512:C0],
                                         start=True, stop=False)
                    nc.tensor.matmul(t1s, lhs, cb[:, C0:CT],
                                     start=True, stop=False)
                    nc.tensor.matmul(sc[:, 0:512], ident_sb[0:ACH, 0:ACH],
                                     A_big[:, h, 0:512], start=False, stop=True)
                    if C0 > 512:
                        nc.tensor.matmul(sc[:, 512:C0], ident_sb[0:ACH, 0:ACH],
                                         A_big[:, h, 512:C0],
                                         start=False, stop=True)
                    nc.tensor.matmul(t1s, ident_sb[0:ACH, 0:ACH],
                                     A_big[:, h, C0:CT], start=False, stop=True)
                    # fused |.|-reduce: [p, 2H, D] -> [p, 2H]
                    nc.vector.tensor_reduce(
                        stage[:, sidx, :],
                        sc[:, 0:C0].rearrange("p (k d) -> p k d", d=D),
                        axis=AX, op=ADD, apply_absolute_value=True)

            # ---- group softmax ----
            gsz = NAH * 8 * H
            score_g = p1_sb.tile([ACH, gsz], f32, tag="score", bufs=4)
            # score = pos - neg  (+ T1)
            nc.vector.tensor_sub(
                score_g[:].rearrange("p (s k) -> p s k", k=H),
                stage[:, :, 0:H],
                stage[:, :, H : 2 * H])
            nc.vector.tensor_add(score_g[:], score_g[:],
                                 misc[0:ACH, T1_OFF : T1_OFF + gsz])
            exp_g = p1_sb.tile([ACH, gsz], f32, tag="expg", bufs=6)
            nc.scalar.activation(
                exp_g[:].rearrange("p (a c b) -> p a b c", a=NAH, c=H),
                score_g[:].rearrange("p (a b c) -> p a b c", a=NAH, b=8),
                EXPF)
            for h in range(NAH):
                nc.tensor.matmul(
                    misc[0:1, SUM_OFF : SUM_OFF + 8 * H], ones_col[0:ACH, :],
                    exp_g[:, h * 8 * H : (h + 1) * 8 * H],
                    start=(h == 0), stop=(h == NAH - 1))
            rec = p1_sb.tile([1, 8 * H], f32, tag="rec", bufs=4)
            nc.vector.reciprocal(rec[:], misc[0:1, SUM_OFF : SUM_OFF + 8 * H])
            nc.tensor.matmul(misc[0:128, RB_OFF : RB_OFF + 8 * H],
                             ones_row[:, 0:128], rec[:], start=True, stop=True)
            # alpha normalized -> bf16 alpha_v3
            for h in range(NAH):
                sl = exp_g[:, h * 8 * H : (h + 1) * 8 * H]
                nc.vector.tensor_mul(sl, sl,
                                     misc[0:ACH, RB_OFF : RB_OFF + 8 * H])
                nc.scalar.copy(
                    alpha_v3[:, h, :, g * 8 : g * 8 + 8],
                    sl.rearrange("p (k u) -> p k u", k=H))
            # ---- user-side weighted edge sums ----
            for ui in range(8):
                u = g * 8 + ui
                for h in range(NAH):
                    nc.tensor.matmul(
                        misc[0:ED, EW_OFF + ui * H : EW_OFF + (ui + 1) * H],
                        nats[ui // 4][:, ui % 4, h, :],
                        alpha_v3[:, h, :, u],
                        start=(h == 0), stop=(h == NAH - 1))
            nc.scalar.copy(
                ew_all[:, g * 8 : g * 8 + 8, :].rearrange("p a b -> p (a b)"),
                misc[0:ED, EW_OFF : EW_OFF + 8 * H])

    # ---------- pass 3: ant-side sums and outputs ----------
    with tc.tile_pool(name="p3_sb", bufs=3) as p3_sb, \
         tc.tile_pool(name="p3_ps", bufs=2, space="PSUM") as p3_ps, \
         tc.tile_pool(name="po_ps", bufs=2, space="PSUM") as po_ps:
        # alpha_v2 via [128,128] transposes of alpha_v3
        for j in range(NUC):
            for k in range(H):
                for h in range(NAH):
                    pt2 = p3_ps.tile([UC, 512], bf16, tag="pt2")
                    nc.tensor.transpose(
                        pt2[:, 0:ACH],
                        alpha_v3[:, h, k, j * UC : (j + 1) * UC],
                        ident_sb[0:ACH, 0:ACH])
                    nc.scalar.copy(
                        alpha_v2[:, j, k, h * ACH : (h + 1) * ACH],
                        pt2[0:UC, 0:ACH])
        # ant-side weighted edge sums (contract over users)
        for ag in range(NA // 8):
            pe = p3_ps.tile([ED, 512], f32, tag="pewa")
            evs = []
            for j in range(NUC):
                ev = p3_sb.tile([UC, 8, ED], bf16, tag="ev", bufs=6)
                nc.sync.dma_start(ev[:], edge_p3[j, :, ag * 8 : ag * 8 + 8, :])
                evs.append(ev)
            for ai in range(8):
                a = ag * 8 + ai
                for j in range(NUC):
                    nc.tensor.matmul(
                        pe[:, ai * H : (ai + 1) * H],
                        evs[j][:, ai, :], alpha_v2[:, j, :, a],
                        start=(j == 0), stop=(j == NUC - 1))
            nc.scalar.copy(
                ewa_all[:, ag * 8 : ag * 8 + 8, :].rearrange("p a b -> p (a b)"),
                pe[:, 0 : 8 * H])
        # user_out = concat_k(alpha@A_k + ew@We_k) + user@Wres
        uo_v = user_out.rearrange("(j p) d -> j p d", p=UC)
        for j in range(NUC):
            po = po_ps.tile([UC, HIDDEN], f32, tag="puo")
            nc.tensor.matmul(po[:], userT[:, j * UC : j * UC + UC],
                             wres_sb[:], start=True, stop=False)
            for k in range(H):
                for h in range(NAH):
                    nc.tensor.matmul(
                        po[:, k * HD : (k + 1) * HD],
                        alpha_v3[:, h, k, j * UC : j * UC + UC],
                        A_big[:, h, CT + k * HD : CT + (k + 1) * HD],
                        start=False, stop=False)
                nc.tensor.matmul(
                    po[:, k * HD : (k + 1) * HD],
                    ew_all[:, j * UC : j * UC + UC, k],
                    we_big_sb[:, CT + k * HD : CT + (k + 1) * HD],
                    start=False, stop=True)
            ob = p3_sb.tile([UC, HIDDEN], f32, tag="ob")
            nc.scalar.copy(ob[:], po[:])
            nc.sync.dma_start(uo_v[j], ob[:])
        # ant_out = concat_k(alpha^T@U_k + ewa@We_k)
        ao_v = ant_out.rearrange("(i p) d -> i p d", p=ACH)
        for i in range(NA // ACH):
            po = po_ps.tile([ACH, HIDDEN], f32, tag="pao")
            for k in range(H):
                for j in range(NUC):
                    nc.tensor.matmul(
                        po[:, k * HD : (k + 1) * HD],
                        alpha_v2[:, j, k, i * ACH : (i + 1) * ACH],
                        U_big[:, j, CT + k * HD : CT + (k + 1) * HD],
                        start=(j == 0), stop=False)
                nc.tensor.matmul(
                    po[:, k * HD : (k + 1) * HD],
                    ewa_all[:, i * ACH : (i + 1) * ACH, k],
                    we_big_sb[:, CT + k * HD : CT + (k + 1) * HD],
                    start=False, stop=True)
            ob = p3_sb.tile([ACH, HIDDEN], f32, tag="ob2")
            nc.scalar.copy(ob[:], po[:])
            nc.sync.dma_start(ao_v[i], ob[:])


# ---------------------------------------------------------------------------
_CACHE = {}


def _get_nc(cfg):
    key = ("nc4", cfg["D"])
    if key in _CACHE:
        return _CACHE[key]
    import concourse.bacc as bacc
    import concourse.mybir as mybir
    import concourse.tile as tile

    f32 = mybir.dt.float32
    bf16 = mybir.dt.bfloat16
    nc = bacc.Bacc("TRN2", target_bir_lowering=False, debug=False)
    NU, NA, ED, UD, AD = cfg["NU"], cfg["NA"], cfg["ED"], cfg["UD"], cfg["AD"]
    CT, HH = cfg["CT"], cfg["HH"]
    ins = {
        "edge": nc.dram_tensor("edge", [NU * NA, ED], bf16, kind="ExternalInput").ap(),
        "user": nc.dram_tensor("user", [NU, UD], bf16, kind="ExternalInput").ap(),
        "ant": nc.dram_tensor("ant", [NA, AD], bf16, kind="ExternalInput").ap(),
        "wu_big": nc.dram_tensor("wu_big", [UD, CT + HH], bf16, kind="ExternalInput").ap(),
        "wa_big": nc.dram_tensor("wa_big", [AD, CT + HH], bf16, kind="ExternalInput").ap(),
        "we_big": nc.dram_tensor("we_big", [ED, CT + HH], bf16, kind="ExternalInput").ap(),
        "wres": nc.dram_tensor("wres", [UD, HH], bf16, kind="ExternalInput").ap(),
        "ident": nc.dram_tensor("ident", [128, 128], bf16, kind="ExternalInput").ap(),
    }
    outs = {
        "user_out": nc.dram_tensor("user_out", [NU, HH], f32, kind="ExternalOutput").ap(),
        "ant_out": nc.dram_tensor("ant_out", [NA, HH], f32, kind="ExternalOutput").ap(),
    }
    with tile.TileContext(nc) as tc:
        with ExitStack() as ctx:
            build_bgat(ctx, tc, outs, ins, cfg)
    nc.finalize()
    _CACHE[key] = nc
    return nc


_LAST_RES = {}


def _to_bf16(a):
    """Fast f32 -> bf16 with round-to-nearest-even via uint16 trick."""
    a = np.ascontiguousarray(a, np.float32)
    u = a.view(np.uint32)
    r = ((u >> 16) + ((u >> 15) & 1)).astype(np.uint16)
    return r.view(BF16).reshape(a.shape)


def kernel(user_feats, ant_feats, edge_feats, Wu, Wa, We, av, Wres,
           _trace=False):
    from concourse.bass_utils import run_bass_kernel_spmd

    cfg = make_cfg(**FULL_CFG, av=av)
    wd = prep_weights(Wu, Wa, We, av, Wres, cfg)
    nc = _get_nc(cfg)
    NU, NA, ED = cfg["NU"], cfg["NA"], cfg["ED"]

    user_b = _to_bf16(np.asarray(user_feats, np.float32))
    ant_b = _to_bf16(np.asarray(ant_feats, np.float32))
    edge_b = _to_bf16(np.asarray(edge_feats, np.float32))

    in_maps = []
    for b in range(B):
        in_maps.append({
            "edge": edge_b[b].reshape(NU * NA, ED),
            "user": user_b[b],
            "ant": ant_b[b],
            "wu_big": wd["wu_big"], "wa_big": wd["wa_big"],
            "we_big": wd["we_big"], "wres": wd["wres"], "ident": wd["ident"],
        })
    res = run_bass_kernel_spmd(nc, in_maps, core_ids=list(range(B)),
                               trace=_trace)
    _LAST_RES["res"] = res
    user_out = np.stack([res.results[b]["user_out"] for b in range(B)])
    ant_out = np.stack([res.results[b]["ant_out"] for b in range(B)])
    return (user_out, ant_out)


# revision 4
# speedup vs baseline: 2.0574x; 1.7115x over previous
"""BGAT attention kernel v2 for Trainium2 (8 NeuronCores, batch-parallel).

Changes vs v1: all matmul operands bf16 (4x PE throughput per the cost
model's 4-cycles-per-row fp32 penalty), edge shipped/loaded as bf16
(half wire + HBM bytes), batched edge DMAs, single fused abs-reduce per
tile with uniform head padding D, ACT-engine combo-row assembly (no
gpsimd SBUF-SBUF DMAs), fattened output matmuls.

Math (per core = one batch element b):
  score[u,a,k] = scale * av_k . lrelu(U_k[u] + A_k[a] + E_k[u,a])
  with lrelu(x) = 0.6x + 0.4|x|:
    score = T1 + sum_pos |S''| - sum_neg |S''|
  S''[., c] columns hold (U+A+E) projected through per-column weights
  folded with 0.4*scale*|av_d|; T1 is the exact linear term.
  E-term rides a K=65 matmul (transposed edge tile + ones row x U-row);
  A-term rides an identity matmul into the same PSUM accumulation.
  alpha = softmax(score) over antennas (no max-subtraction needed:
  scores are tiny by construction).
  Messages contract edge with alpha BEFORE projecting:
    sum_a alpha*E = (sum_a alpha*edge) @ We  (same over u).
"""

import math
from contextlib import ExitStack

import numpy as np
import ml_dtypes

BF16 = ml_dtypes.bfloat16

# ---- problem sizes (hardcoded from spec) ----
B = 8
FULL_CFG = dict(NU=256, NA=256, ED=64, UD=128, AD=128, H=8, HD=64)
SLOPE = 0.2


def make_cfg(NU, NA, ED, UD, AD, H, HD, av):
    cfg = dict(NU=NU, NA=NA, ED=ED, UD=UD, AD=AD, H=H, HD=HD)
    cfg["HH"] = H * HD
    scale = 1.0 / math.sqrt(HD)
    av = np.asarray(av, np.float32)
    pos_idx = [np.nonzero(av[k] >= 0)[0] for k in range(H)]
    neg_idx = [np.nonzero(av[k] < 0)[0] for k in range(H)]
    D = max(max(len(ix) for ix in pos_idx), max(len(ix) for ix in neg_idx))
    cfg["D"] = D
    cfg["C0"] = 2 * H * D          # score cols: pos block | neg block
    cfg["CT"] = cfg["C0"] + H      # + T1 cols
    cfg["pos_idx"], cfg["neg_idx"] = pos_idx, neg_idx
    cfg["scale"] = scale
    cfg["NAH"] = (NA + 127) // 128
    cfg["ACH"] = min(128, NA)
    cfg["UC"] = min(128, NU)
    cfg["NUC"] = NU // cfg["UC"]
    assert NU % 8 == 0
    cfg["NG"] = NU // 8
    return cfg


def prep_weights(Wu, Wa, We, av, Wres, cfg):
    """Folded/permuted weight blocks in bf16. Column layout (CT+HH wide):
    [k*D+i : pos] [H*D + k*D+i : neg] [2*H*D + k : T1] [CT + k*HD+d : raw]"""
    H, HD, ED, UD, AD = cfg["H"], cfg["HD"], cfg["ED"], cfg["UD"], cfg["AD"]
    D, C0, CT, HH = cfg["D"], cfg["C0"], cfg["CT"], cfg["HH"]
    scale = cfg["scale"]
    Wu, Wa, We = (np.asarray(x, np.float32) for x in (Wu, Wa, We))
    av = np.asarray(av, np.float32)
    Wres = np.asarray(Wres, np.float32)

    wu_big = np.zeros((UD, CT + HH), np.float32)
    wa_big = np.zeros((AD, CT + HH), np.float32)
    we_big = np.zeros((ED, CT + HH), np.float32)
    for k in range(H):
        for i, d in enumerate(cfg["pos_idx"][k]):
            c = 0.4 * scale * abs(av[k, d])
            col = k * D + i
            wu_big[:, col] = Wu[k][:, d] * c
            wa_big[:, col] = Wa[k][:, d] * c
            we_big[:, col] = We[k][:, d] * c
        for i, d in enumerate(cfg["neg_idx"][k]):
            c = 0.4 * scale * abs(av[k, d])
            col = H * D + k * D + i
            wu_big[:, col] = Wu[k][:, d] * c
            wa_big[:, col] = Wa[k][:, d] * c
            we_big[:, col] = We[k][:, d] * c
        t1w = 0.6 * scale * av[k]
        col = C0 + k
        wu_big[:, col] = Wu[k] @ t1w
        wa_big[:, col] = Wa[k] @ t1w
        we_big[:, col] = We[k] @ t1w
        wu_big[:, CT + k * HD : CT + (k + 1) * HD] = Wu[k]
        wa_big[:, CT + k * HD : CT + (k + 1) * HD] = Wa[k]
        we_big[:, CT + k * HD : CT + (k + 1) * HD] = We[k]

    ident = np.eye(128, dtype=np.float32)
    return dict(wu_big=wu_big.astype(BF16), wa_big=wa_big.astype(BF16),
                we_big=we_big.astype(BF16), wres=Wres.astype(BF16),
                ident=ident.astype(BF16))


def build_bgat(ctx: ExitStack, tc, outs, ins, cfg):
    import concourse.bass as bass
    import concourse.mybir as mybir

    nc = tc.nc
    f32 = mybir.dt.float32
    bf16 = mybir.dt.bfloat16
    AX = mybir.AxisListType.X
    ADD = mybir.AluOpType.add
    EXPF = mybir.ActivationFunctionType.Exp

    NU, NA, ED, UD, AD = cfg["NU"], cfg["NA"], cfg["ED"], cfg["UD"], cfg["AD"]
    H, HD, HH = cfg["H"], cfg["HD"], cfg["HH"]
    D, C0, CT = cfg["D"], cfg["C0"], cfg["CT"]
    NAH, ACH, UC, NUC, NG = cfg["NAH"], cfg["ACH"], cfg["UC"], cfg["NUC"], cfg["NG"]
    HIDDEN = HH

    edge = ins["edge"]      # [NU*NA, ED] bf16
    user = ins["user"]      # [NU, UD] bf16
    ant = ins["ant"]        # [NA, AD] bf16
    wu_big_d = ins["wu_big"]
    wa_big_d = ins["wa_big"]
    we_big_d = ins["we_big"]
    wres_d = ins["wres"]
    ident_d = ins["ident"]
    user_out = outs["user_out"]  # [NU, HIDDEN] f32
    ant_out = outs["ant_out"]    # [NA, HIDDEN] f32

    # edge views
    # pass-1: per user u, antenna-major tiles [128a, 64e]
    edge_p1 = edge.rearrange("(u h p) e -> p u h e", h=NAH, p=ACH)
    # pass-3: user-major slabs [128u, 8a, 64e]
    edge_p3 = edge.rearrange("(j p a) e -> j p a e", p=UC, a=NA)

    consts = ctx.enter_context(tc.tile_pool(name="consts", bufs=1))

    # ---------- persistent SBUF tensors ----------
    ident_sb = consts.tile([128, 128], bf16)
    nc.sync.dma_start(ident_sb[:], ident_d[:, :])
    wu_big_sb = consts.tile([UD, CT + HH], bf16)
    nc.sync.dma_start(wu_big_sb[:], wu_big_d[:, :])
    wa_big_sb = consts.tile([AD, CT + HH], bf16)
    nc.sync.dma_start(wa_big_sb[:], wa_big_d[:, :])
    we_big_sb = consts.tile([ED, CT + HH], bf16)
    nc.sync.dma_start(we_big_sb[:], we_big_d[:, :])
    wres_sb = consts.tile([UD, HIDDEN], bf16)
    nc.sync.dma_start(wres_sb[:], wres_d[:, :])

    ones_col = consts.tile([128, 1], f32)
    nc.gpsimd.memset(ones_col[:], 1.0)
    ones_row = consts.tile([1, 128], f32)
    nc.gpsimd.memset(ones_row[:], 1.0)

    U_big = consts.tile([UC, NUC, CT + HH], bf16)
    A_big = consts.tile([ACH, NAH, CT + HH], bf16)
    userT = consts.tile([UD, NU], bf16)
    antT = consts.tile([AD, NA], bf16)
    alpha_v3 = consts.tile([ACH, NAH, H, NU], bf16)   # [a, h, k, u]
    alpha_v2 = consts.tile([UC, NUC, H, NA], bf16)    # [u, j, k, a]
    ew_all = consts.tile([ED, NU, H], bf16)
    ewa_all = consts.tile([ED, NA, H], bf16)
    # full u-major edge copy, prefetched during pass 0/1 (no deps)
    ev_all = consts.tile([UC, NUC, NA, ED], bf16)
    for j in range(NUC):
        nc.sync.dma_start(ev_all[:, j, :, :], edge_p3[j])

    # edT ring: transposed edge tiles [65, 4*128] (4 chunks each), row 64=1
    N_EDT = 8
    edT_ring = []
    for i in range(N_EDT):
        edT_i = consts.tile([ED + 1, 512], bf16, tag=f"edT{i}", name=f"edT{i}")
        edT_ring.append(edT_i)
    for t in edT_ring:
        nc.vector.memset(t[ED : ED + 1, :], 1.0)
    # combo ring: [65, CT]; rows 0:64 = we_big[:, 0:CT] const, row 64 = U row
    N_CB = 4
    combos = []
    for i in range(N_CB):
        cb_i = consts.tile([ED + 1, CT], bf16, tag=f"cb{i}", name=f"cb{i}")
        combos.append(cb_i)
    for cb in combos:
        nc.sync.dma_start(cb[0:ED, :], we_big_d[:, 0:CT])

    # ---------- pass 0: transposes and U/A projections ----------
    with tc.tile_pool(name="pre_sb", bufs=2) as pre_sb, \
         tc.tile_pool(name="pre_ps", bufs=2, space="PSUM") as pre_ps:
        for (feat, T_sb, n, fd) in ((user, userT, NU, UD), (ant, antT, NA, AD)):
            fv = feat.rearrange("(j p) f -> j p f", p=min(128, n))
            for j in range(fv.shape[0]):
                p = fv.shape[1]
                ft = pre_sb.tile([p, fd], bf16, tag="ft")
                nc.sync.dma_start(ft[:], fv[j])
                pt = pre_ps.tile([fd, p], bf16, tag="pt")
                nc.tensor.transpose(pt[:], ft[:], ident_sb[0:p, 0:p])
                nc.scalar.copy(T_sb[:, j * p : j * p + p], pt[:])
        for (T_sb, big, nchunk, pc, fd) in (
            (userT, U_big, NUC, UC, UD),
            (antT, A_big, NAH, ACH, AD),
        ):
            w_sb = wu_big_sb if big is U_big else wa_big_sb
            for j in range(nchunk):
                for c0 in range(0, CT + HH, 512):
                    c1 = min(c0 + 512, CT + HH)
                    ps = pre_ps.tile([pc, 512], f32, tag="proj")
                    nc.tensor.matmul(ps[:, 0 : c1 - c0],
                                     T_sb[:, j * pc : j * pc + pc],
                                     w_sb[:, c0:c1], start=True, stop=True)
                    nc.scalar.copy(big[:, j, c0:c1], ps[:, 0 : c1 - c0])

    # ---------- pass 1: scores + softmax + user-side weighted edge sums ----
    # misc psum bank layout (per group of 8 users):
    T1_OFF = 0                       # [128, 16*8]
    SUM_OFF = T1_OFF + NAH * 8 * H   # [1, 64]
    RB_OFF = SUM_OFF + 8 * H         # [128, 64]
    EW_OFF = RB_OFF + 8 * H          # [64, 64]
    assert EW_OFF + 8 * H <= 512

    n_tp_batches = 0
    with tc.tile_pool(name="nat_pool", bufs=6) as nat_pool, \
         tc.tile_pool(name="p1_sb", bufs=3) as p1_sb, \
         tc.tile_pool(name="p1_stage", bufs=2) as p1_stage, \
         tc.tile_pool(name="ps_score", bufs=2, space="PSUM") as ps_score_pool, \
         tc.tile_pool(name="ps_tp", bufs=2, space="PSUM") as ps_tp_pool, \
         tc.tile_pool(name="ps_misc", bufs=2, space="PSUM") as ps_misc_pool:

        for g in range(NG):
            misc = ps_misc_pool.tile([128, 512], f32, tag="misc")
            stage = p1_stage.tile([ACH, NAH * 8, 2 * H], f32, tag="st")
            # natural-layout edge tiles for this group: 2 DMAs x 4 users
            nats = []
            for q in range(2):
                natt = nat_pool.tile([ACH, 4, NAH, ED], bf16, tag="nat")
                u0 = g * 8 + q * 4
                nc.sync.dma_start(natt[:], edge_p1[:, u0 : u0 + 4])
                nats.append(natt)

            chunks = [(ui, h) for ui in range(8) for h in range(NAH)]
            for u4 in range(0, len(chunks), 4):
                batch = chunks[u4 : u4 + 4]
                edT = edT_ring[n_tp_batches % N_EDT]
                n_tp_batches += 1
                tp = ps_tp_pool.tile([ED, 512], bf16, tag="tp")
                for q, (ui, h) in enumerate(batch):
                    nc.tensor.transpose(
                        tp[:, q * 128 : q * 128 + ACH],
                        nats[ui // 4][:, ui % 4, h, :],
                        ident_sb[0:ACH, 0:ACH])
                for q, (ui, h) in enumerate(batch):
                    if h == 0:
                        u = g * 8 + ui
                        cb = combos[u % N_CB]
                        nc.sync.dma_start(
                            cb[ED : ED + 1, :],
                            U_big[u % UC : u % UC + 1, u // UC, 0:CT])
                nc.scalar.copy(edT[0:ED, :], tp[:, :])
                for q, (ui, h) in enumerate(batch):
                    u = g * 8 + ui
                    cb = combos[u % N_CB]
                    sidx = h * 8 + ui
                    lhs = edT[0 : ED + 1, q * 128 : q * 128 + ACH]
                    sc = ps_score_pool.tile([ACH, C0], f32, tag="sc")
                    t1s = misc[0:ACH, T1_OFF + sidx * H : T1_OFF + (sidx + 1) * H]
                    # E+U into psum (K=65 augmented), then A via identity mm
                    nc.tensor.matmul(sc[:, 0:512], lhs, cb[:, 0:512],
                                     start=True, stop=False)
                    if C0 > 512:
                        nc.tensor.matmul(sc[:, 512:C0], lhs, cb[:, 512:C0],
                                         start=True, stop=False)
                    nc.tensor.matmul(t1s, lhs, cb[:, C0:CT],
                                     start=True, stop=False)
                    nc.tensor.matmul(sc[:, 0:512], ident_sb[0:ACH, 0:ACH],
                                     A_big[:, h, 0:512], start=False, stop=True)
                    if C0 > 512:
                        nc.tensor.matmul(sc[:, 512:C0], ident_sb[0:ACH, 0:ACH],
                                         A_big[:, h, 512:C0],
                                         start=False, stop=True)
                    nc.tensor.matmul(t1s, ident_sb[0:ACH, 0:ACH],
                                     A_big[:, h, C0:CT], start=False, stop=True)
                    # fused |.|-reduce: [p, 2H, D] -> [p, 2H]
                    nc.vector.tensor_reduce(
                        stage[:, sidx, :],
                        sc[:, 0:C0].rearrange("p (k d) -> p k d", d=D),
                        axis=AX, op=ADD, apply_absolute_value=True)

            # ---- group softmax ----
            gsz = NAH * 8 * H
            score_g = p1_sb.tile([ACH, gsz], f32, tag="score", bufs=4)
            # score = pos - neg  (+ T1)
            nc.vector.tensor_sub(
                score_g[:].rearrange("p (s k) -> p s k", k=H),
                stage[:, :, 0:H],
                stage[:, :, H : 2 * H])
            nc.vector.tensor_add(score_g[:], score_g[:],
                                 misc[0:ACH, T1_OFF : T1_OFF + gsz])
            exp_g = p1_sb.tile([ACH, gsz], f32, tag="expg", bufs=6)
            nc.scalar.activation(
                exp_g[:].rearrange("p (a c b) -> p a b c", a=NAH, c=H),
                score_g[:].rearrange("p (a b c) -> p a b c", a=NAH, b=8),
                EXPF)
            for h in range(NAH):
                nc.tensor.matmul(
                    misc[0:1, SUM_OFF : SUM_OFF + 8 * H], ones_col[0:ACH, :],
                    exp_g[:, h * 8 * H : (h + 1) * 8 * H],
                    start=(h == 0), stop=(h == NAH - 1))
            rec = p1_sb.tile([1, 8 * H], f32, tag="rec", bufs=4)
            nc.vector.reciprocal(rec[:], misc[0:1, SUM_OFF : SUM_OFF + 8 * H])
            nc.tensor.matmul(misc[0:128, RB_OFF : RB_OFF + 8 * H],
                             ones_row[:, 0:128], rec[:], start=True, stop=True)
            # alpha normalized -> bf16 alpha_v3
            for h in range(NAH):
                sl = exp_g[:, h * 8 * H : (h + 1) * 8 * H]
                nc.vector.tensor_mul(sl, sl,
                                     misc[0:ACH, RB_OFF : RB_OFF + 8 * H])
                nc.scalar.copy(
                    alpha_v3[:, h, :, g * 8 : g * 8 + 8],
                    sl.rearrange("p (k u) -> p k u", k=H))
            # ---- user-side weighted edge sums ----
            for ui in range(8):
                u = g * 8 + ui
                for h in range(NAH):
                    nc.tensor.matmul(
                        misc[0:ED, EW_OFF + ui * H : EW_OFF + (ui + 1) * H],
                        nats[ui // 4][:, ui % 4, h, :],
                        alpha_v3[:, h, :, u],
                        start=(h == 0), stop=(h == NAH - 1))
            nc.scalar.copy(
                ew_all[:, g * 8 : g * 8 + 8, :].rearrange("p a b -> p (a b)"),
                misc[0:ED, EW_OFF : EW_OFF + 8 * H])

    # ---------- pass 3: ant-side sums and outputs ----------
    with tc.tile_pool(name="p3_sb", bufs=3) as p3_sb, \
         tc.tile_pool(name="p3_ps", bufs=2, space="PSUM") as p3_ps, \
         tc.tile_pool(name="po_ps", bufs=2, space="PSUM") as po_ps:
        # user_out = concat_k(alpha@A_k + ew@We_k) + user@Wres
        uo_v = user_out.rearrange("(j p) d -> j p d", p=UC)
        for j in range(NUC):
            po = po_ps.tile([UC, HIDDEN], f32, tag="puo")
            nc.tensor.matmul(po[:], userT[:, j * UC : j * UC + UC],
                             wres_sb[:], start=True, stop=False)
            for k in range(H):
                for h in range(NAH):
                    nc.tensor.matmul(
                        po[:, k * HD : (k + 1) * HD],
                        alpha_v3[:, h, k, j * UC : j * UC + UC],
                        A_big[:, h, CT + k * HD : CT + (k + 1) * HD],
                        start=False, stop=False)
                nc.tensor.matmul(
                    po[:, k * HD : (k + 1) * HD],
                    ew_all[:, j * UC : j * UC + UC, k],
                    we_big_sb[:, CT + k * HD : CT + (k + 1) * HD],
                    start=False, stop=True)
            ob = p3_sb.tile([UC, HIDDEN], f32, tag="ob")
            nc.scalar.copy(ob[:], po[:])
            nc.sync.dma_start(uo_v[j], ob[:])
        # alpha_v2 via [128,128] transposes of alpha_v3
        for j in range(NUC):
            for k in range(H):
                for h in range(NAH):
                    pt2 = p3_ps.tile([UC, 512], bf16, tag="pt2")
                    nc.tensor.transpose(
                        pt2[:, 0:ACH],
                        alpha_v3[:, h, k, j * UC : (j + 1) * UC],
                        ident_sb[0:ACH, 0:ACH])
                    nc.scalar.copy(
                        alpha_v2[:, j, k, h * ACH : (h + 1) * ACH],
                        pt2[0:UC, 0:ACH])
        # ant-side weighted edge sums (contract over users)
        for ag in range(NA // 8):
            pe = p3_ps.tile([ED, 512], f32, tag="pewa")
            for ai in range(8):
                a = ag * 8 + ai
                for j in range(NUC):
                    nc.tensor.matmul(
                        pe[:, ai * H : (ai + 1) * H],
                        ev_all[:, j, a, :], alpha_v2[:, j, :, a],
                        start=(j == 0), stop=(j == NUC - 1))
            nc.scalar.copy(
                ewa_all[:, ag * 8 : ag * 8 + 8, :].rearrange("p a b -> p (a b)"),
                pe[:, 0 : 8 * H])
        # ant_out = concat_k(alpha^T@U_k + ewa@We_k)
        ao_v = ant_out.rearrange("(i p) d -> i p d", p=ACH)
        for i in range(NA // ACH):
            po = po_ps.tile([ACH, HIDDEN], f32, tag="pao")
            for k in range(H):
                for j in range(NUC):
                    nc.tensor.matmul(
                        po[:, k * HD : (k + 1) * HD],
                        alpha_v2[:, j, k, i * ACH : (i + 1) * ACH],
                        U_big[:, j, CT + k * HD : CT + (k + 1) * HD],
                        start=(j == 0), stop=False)
                nc.tensor.matmul(
                    po[:, k * HD : (k + 1) * HD],
                    ewa_all[:, i * ACH : (i + 1) * ACH, k],
                    we_big_sb[:, CT + k * HD : CT + (k + 1) * HD],
                    start=False, stop=True)
            ob = p3_sb.tile([ACH, HIDDEN], f32, tag="ob2")
            nc.scalar.copy(ob[:], po[:])
            nc.sync.dma_start(ao_v[i], ob[:])


# ---------------------------------------------------------------------------
_CACHE = {}


def _get_nc(cfg):
    key = ("nc5", cfg["D"])
    if key in _CACHE:
        return _CACHE[key]
    import concourse.bacc as bacc
    import concourse.mybir as mybir
    import concourse.tile as tile

    f32 = mybir.dt.float32
    bf16 = mybir.dt.bfloat16
    nc = bacc.Bacc("TRN2", target_bir_lowering=False, debug=False)
    NU, NA, ED, UD, AD = cfg["NU"], cfg["NA"], cfg["ED"], cfg["UD"], cfg["AD"]
    CT, HH = cfg["CT"], cfg["HH"]
    ins = {
        "edge": nc.dram_tensor("edge", [NU * NA, ED], bf16, kind="ExternalInput").ap(),
        "user": nc.dram_tensor("user", [NU, UD], bf16, kind="ExternalInput").ap(),
        "ant": nc.dram_tensor("ant", [NA, AD], bf16, kind="ExternalInput").ap(),
        "wu_big": nc.dram_tensor("wu_big", [UD, CT + HH], bf16, kind="ExternalInput").ap(),
        "wa_big": nc.dram_tensor("wa_big", [AD, CT + HH], bf16, kind="ExternalInput").ap(),
        "we_big": nc.dram_tensor("we_big", [ED, CT + HH], bf16, kind="ExternalInput").ap(),
        "wres": nc.dram_tensor("wres", [UD, HH], bf16, kind="ExternalInput").ap(),
        "ident": nc.dram_tensor("ident", [128, 128], bf16, kind="ExternalInput").ap(),
    }
    outs = {
        "user_out": nc.dram_tensor("user_out", [NU, HH], f32, kind="ExternalOutput").ap(),
        "ant_out": nc.dram_tensor("ant_out", [NA, HH], f32, kind="ExternalOutput").ap(),
    }
    with tile.TileContext(nc) as tc:
        with ExitStack() as ctx:
            build_bgat(ctx, tc, outs, ins, cfg)
    nc.finalize()
    _CACHE[key] = nc
    return nc


_LAST_RES = {}


def _to_bf16(a):
    """Fast f32 -> bf16 with round-to-nearest-even via uint16 trick."""
    a = np.ascontiguousarray(a, np.float32)
    u = a.view(np.uint32)
    r = ((u >> 16) + ((u >> 15) & 1)).astype(np.uint16)
    return r.view(BF16).reshape(a.shape)


def kernel(user_feats, ant_feats, edge_feats, Wu, Wa, We, av, Wres,
           _trace=False):
    from concourse.bass_utils import run_bass_kernel_spmd

    cfg = make_cfg(**FULL_CFG, av=av)
    wd = prep_weights(Wu, Wa, We, av, Wres, cfg)
    nc = _get_nc(cfg)
    NU, NA, ED = cfg["NU"], cfg["NA"], cfg["ED"]

    user_b = _to_bf16(np.asarray(user_feats, np.float32))
    ant_b = _to_bf16(np.asarray(ant_feats, np.float32))
    edge_b = _to_bf16(np.asarray(edge_feats, np.float32))

    in_maps = []
    for b in range(B):
        in_maps.append({
            "edge": edge_b[b].reshape(NU * NA, ED),
            "user": user_b[b],
            "ant": ant_b[b],
            "wu_big": wd["wu_big"], "wa_big": wd["wa_big"],
            "we_big": wd["we_big"], "wres": wd["wres"], "ident": wd["ident"],
        })
    res = run_bass_kernel_spmd(nc, in_maps, core_ids=list(range(B)),
                               trace=_trace)
    _LAST_RES["res"] = res
    user_out = np.stack([res.results[b]["user_out"] for b in range(B)])
    ant_out = np.stack([res.results[b]["ant_out"] for b in range(B)])
    return (user_out, ant_out)
